# revision 70
# baseline (speedup 1.0000x reference)
"""Depth-aware 3x3 convolution on 8 Trainium2 NeuronCores (Bass, raw engine blocks).

out[b,o,h,w] = sum_{c,kh,kw} weight[o,c,kh,kw] * x[b,c,h+kh-1,w+kw-1]
                             * exp(-8.3*|depth[b,h,w] - depth[b,h+kh-1,w+kw-1]|)

Sharding: core = 2*b + (h >= 128); each core computes a [32, 128, 256] output
slab from a 130-row padded input frame (1-row halo from the host slice).

v4 pipeline (bf16 datapath, DMA spread across SP + GpSimd + ACT queues,
triple-buffered loads, per-t sim granularity):
  A. sim: 3 row-view depth loads; per-t merged DVE sub+abs, per-t ACT exp,
     per-t DMA store -> DRAM simd[9, 32768] bf16.
  B. main loop over 8 tiles of 4096 px (16 rows):
     - 1 DMA (queue A): x chunk loaded 3x at flat offsets j-1 into the three
       partition groups of x3c [96, 18*258] bf16 (column shifts for free).
     - 3 DMAs (part t=0 on queue A, t=1,2 on queue B): simd rows {3t+j} tile
       window replicated to 32 partitions per j group -> simrep [96, 3*4096].
     - DVE: xm3[:, t] = x3c rows(t..t+16) * simrep(t)  (bf16 2x)  t=0,1,2
     - PE : psum[32@g, 4096] += w3[:, t].T @ xm3[:, t]  (K=96, N=512 x8, bf16)
     - ACT: psum -> out_sb bf16, then ACT-issued DMA out.
  Queue A = SP for even tiles, GpSimd for odd (B is the other one).
"""
import sys

import numpy as np

sys.path.insert(0, "/opt/trn_rl_repo")

import concourse.bass as bass
import concourse.mybir as mybir
from concourse.bass_utils import run_bass_kernel_spmd

F32 = mybir.dt.float32
BF16 = mybir.dt.bfloat16
EXP = mybir.ActivationFunctionType.Exp
COPY = mybir.ActivationFunctionType.Copy

B, C, H, W = 4, 32, 256, 256
O = 32
ALPHA = 8.3
R = 128  # output rows per core
WP = W + 2  # padded width
FR = R + 2  # frame rows per core
NPIX = R * W  # 32768
XLEN = FR * WP + 2  # flat x frame + 1-elem guard pads on both ends
TROWS = 16  # rows per tile
TILE = TROWS * W  # 4096
NT = R // TROWS  # 8
CH_ROWS = TROWS + 2  # x chunk rows
CH_FREE = CH_ROWS * WP  # 4644
MMN = 512  # matmul free-dim chunk (one PSUM bank)
QN = TILE // MMN  # 8
T3 = 3 * TILE
NBL = 3  # load-side buffers (x3c, simrep)


def build_nc():
    nc = bass.Bass("TRN2", target_bir_lowering=False, debug=False, num_devices=8)
    xb_in = nc.declare_dram_parameter("xb", [C, XLEN], BF16, isOutput=False)
    dp_in = nc.declare_dram_parameter("dp", [FR, WP], F32, isOutput=False)
    w3_in = nc.declare_dram_parameter("w3", [96, 96], BF16, isOutput=False)
    # output repacked to 64 partitions: row hq*32+o, col i*2048+pix
    out_d = nc.declare_dram_parameter("outd", [2 * O, NT * 2048], BF16, isOutput=True)
    simd = nc.dram_tensor("simd", [9, NPIX], BF16)

    from contextlib import ExitStack

    ctx = ExitStack()
    with ctx:
        d_sb = ctx.enter_context(nc.sbuf_tensor([128, 3 * WP], F32))
        adiff9 = ctx.enter_context(nc.sbuf_tensor([128, 9 * W], F32))
        sim9 = ctx.enter_context(nc.sbuf_tensor([128, 9 * W], BF16))
        w3_sb = ctx.enter_context(nc.sbuf_tensor([96, 96], BF16))
        x3c = ctx.enter_context(nc.sbuf_tensor([96, NBL * CH_FREE], BF16))
        simrep = ctx.enter_context(nc.sbuf_tensor([96, NBL * T3], BF16))
        xm3 = ctx.enter_context(nc.sbuf_tensor([96, 2 * T3], BF16))
        out_sb = ctx.enter_context(nc.sbuf_tensor([64, 2 * 2048], BF16))
        scr = ctx.enter_context(nc.sbuf_tensor([1, 2], BF16))
        psum = ctx.enter_context(nc.psum_tensor([64, TILE], F32))
        ld_d = ctx.enter_context(nc.semaphore("ld_d"))
        ld_w = ctx.enter_context(nc.semaphore("ld_w"))
        x_e = ctx.enter_context(nc.semaphore("x_e"))
        x_o = ctx.enter_context(nc.semaphore("x_o"))
        sim_dve = ctx.enter_context(nc.semaphore("sim_dve"))
        act_exp = ctx.enter_context(nc.semaphore("act_exp"))
        sim_st = ctx.enter_context(nc.semaphore("sim_st"))
        b0_e = ctx.enter_context(nc.semaphore("b0_e"))
        b0_o = ctx.enter_context(nc.semaphore("b0_o"))
        b12_e = ctx.enter_context(nc.semaphore("b12_e"))
        b12_o = ctx.enter_context(nc.semaphore("b12_o"))
        b2a = ctx.enter_context(nc.semaphore("b2a"))
        cp_p = ctx.enter_context(nc.semaphore("cp_p"))
        mod_sem = ctx.enter_context(nc.semaphore("mod_sem"))
        mod_t = ctx.enter_context(nc.semaphore("mod_t"))
        pe_sem = ctx.enter_context(nc.semaphore("pe_sem"))
        act_cp = ctx.enter_context(nc.semaphore("act_cp"))
        st_e = ctx.enter_context(nc.semaphore("st_e"))
        st_o = ctx.enter_context(nc.semaphore("st_o"))
        block = ctx.enter_context(nc.Block())

        def rap(base_ap, offset, dims):
            return bass.AP(tensor=base_ap.tensor, offset=offset, ap=dims)

        def x3c_load(eng, i):
            # one DMA: 3 column shifts x 32 channels on partitions
            bl = i % NBL
            if i >= 1:
                # self-wait: prior completions of this sem have landed
                eng.wait_ge(x_e, 16 * i)
            src = rap(
                xb_in.ap(),
                16 * i * WP,
                [[1, 3], [XLEN, C], [1, CH_FREE]],
            )
            eng.dma_start(
                x3c[:, bl * CH_FREE : (bl + 1) * CH_FREE], src
            ).then_inc(x_e, 16)

        def brc_load(eng, i, t):
            # simd rows {3t, 3t+1, 3t+2} tile window replicated to 32
            # partitions per j group
            bl = i % NBL
            if t == 0:
                sem = b0_e
                if i >= 1:
                    eng.wait_ge(sem, 16 * i)
            else:
                sem = b12_e if i % 2 == 0 else b12_o
                if i >= 2 and t == 1:
                    # even tiles: tile 0 contributed only 16 (part2 on ACT)
                    prior = 32 * (i // 2) - (16 if i % 2 == 0 else 0)
                    eng.wait_ge(sem, prior)
            src = rap(
                simd.ap(),
                3 * t * NPIX + i * TILE,
                [[NPIX, 3], [0, C], [1, TILE]],
            )
            eng.dma_start(
                simrep[:, bl * T3 + t * TILE : bl * T3 + (t + 1) * TILE],
                src,
            ).then_inc(sem, 16)

        def reuse_wait(eng, i):
            # x3c/simrep buffer i%NBL was consumed by tile i-NBL's mul(s);
            # tile 0 is per-t (mod_t), tiles 1..NT-3 are merged (mod_sem)
            if i == NBL:
                eng.wait_ge(mod_t, 3)
            elif i > NBL:
                eng.wait_ge(mod_sem, i - NBL)

        @block.sync
        def _(sync: bass.BassEngine):
            # depth views: order 1,0,2 (sub_t=0 needs views 0 and 1)
            for t in (1, 0, 2):
                sync.dma_start(
                    d_sb[:, t * WP : (t + 1) * WP], dp_in[t : t + 128, :]
                ).then_inc(ld_d, 16)
            sync.dma_start(w3_sb[:], w3_in[:]).then_inc(ld_w, 16)
            x3c_load(sync, 0)
            x3c_load(sync, 1)
            sync.wait_ge(sim_st, 16)
            brc_load(sync, 0, 0)
            sync.wait_ge(sim_st, 32)
            brc_load(sync, 0, 1)
            brc_load(sync, 1, 0)
            for i in range(2, NT):
                reuse_wait(sync, i)
                x3c_load(sync, i)
                brc_load(sync, i, 0)
                if i % 2 == 0:
                    brc_load(sync, i, 1)
                    brc_load(sync, i, 2)


        @block.vector
        def _(vector):
            # sim phase: per-tap diff (DVE stride-0 broadcast is not
            # HW-supported), merged per-t abs
            ad_ap = adiff9[:, 0:1]
            vector.wait_ge(ld_d, 48)
            for t in range(3):
                for j in range(3):
                    k = 3 * t + j
                    vector.tensor_sub(
                        adiff9[:, k * W : (k + 1) * W],
                        d_sb[:, WP + 1 : WP + 1 + W],
                        d_sb[:, t * WP + j : t * WP + j + W],
                    )
                vector.drain()
                av = rap(ad_ap, 3 * t * W, [[9 * W, 128], [W, 3], [1, W]])
                vector.scalar_tensor_tensor(
                    av,
                    av,
                    -1.0,
                    av,
                    op0=mybir.AluOpType.mult,
                    op1=mybir.AluOpType.max,
                ).then_inc(sim_dve, 1)
            # drain assist: hq1 psum->sbuf copies for the last two tiles
            # (DVE is idle once its last mul retires; GpSimd cannot touch
            # PSUM on real HW)
            def drain_copies():
                for i in (NT - 2, NT - 1):
                    sb = i % 2
                    g = i % 2
                    vector.wait_ge(pe_sem, i + 1)
                    vector.wait_ge(st_e if sb == 0 else st_o, 16 * (i // 2))
                    vector.tensor_copy(
                        out_sb[32:64, sb * 2048 : (sb + 1) * 2048],
                        psum[32 * g : 32 * (g + 1), 2048:4096],
                    ).then_inc(cp_p, 1)

            # modulation loop: merged mul per tile; tiles 0, NT-2, NT-1 are
            # split per-t (faster fill and drain)
            xm_ap = xm3[:, 0:1]
            sr_ap = simrep[:, 0:1]
            x3_ap = x3c[:, 0:1]
            split = (0, NT - 2, NT - 1)
            nsplit = 0
            for i in range(NT):
                bl = i % NBL
                sb = i % 2
                half = i // 2 + 1
                vector.wait_ge(x_e, 16 * (i + 1))
                vector.wait_ge(b0_e, 16 * (i + 1))
                if i == 0:
                    pass  # b12/b2a waits interleaved per t below
                elif sb == 0:
                    vector.wait_ge(b12_e, 16 + 32 * (i // 2))
                else:
                    vector.wait_ge(b12_o, 32 * half)
                if i >= 2:
                    vector.wait_ge(pe_sem, i - 1)
                if i not in split:
                    din = [[2 * T3, 96], [TILE, 3], [W, TROWS], [1, W]]
                    vector.tensor_mul(
                        rap(xm_ap, sb * T3, din),
                        rap(
                            x3_ap,
                            bl * CH_FREE + 1,
                            [[NBL * CH_FREE, 96], [WP, 3], [WP, TROWS], [1, W]],
                        ),
                        rap(sr_ap, bl * T3, [[NBL * T3, 96]] + din[1:]),
                    ).then_inc(mod_sem, 1)
                else:
                    dt1 = [[2 * T3, 96], [W, TROWS], [1, W]]
                    for t in range(3):
                        if i == 0 and t == 1:
                            vector.wait_ge(b12_e, 16)
                        if i == 0 and t == 2:
                            vector.wait_ge(b2a, 16)
                        vector.tensor_mul(
                            rap(xm_ap, sb * T3 + t * TILE, dt1),
                            rap(
                                x3_ap,
                                bl * CH_FREE + t * WP + 1,
                                [[NBL * CH_FREE, 96], [WP, TROWS], [1, W]],
                            ),
                            rap(
                                sr_ap,
                                bl * T3 + t * TILE,
                                [[NBL * T3, 96]] + dt1[1:],
                            ),
                        ).then_inc(mod_t, 1)
                    nsplit += 1
            drain_copies()

        @block.tensor
        def _(tensor):
            tensor.wait_ge(ld_w, 16)
            split = (0, NT - 2, NT - 1)
            nsplit = 0
            for i in range(NT):
                sb = i % 2
                g = i % 2
                if i >= 2:
                    tensor.wait_ge(act_cp, i - 1)
                if i not in split:
                    tensor.wait_ge(mod_sem, i)
                for t in range(3):
                    if i in split:
                        tensor.wait_ge(mod_t, 3 * nsplit + t + 1)
                    for q in range(QN):
                        mm = tensor.matmul(
                            psum[32 * g : 32 * (g + 1), q * MMN : (q + 1) * MMN],
                            w3_sb[:, 32 * t : 32 * (t + 1)],
                            xm3[
                                :,
                                sb * T3
                                + t * TILE
                                + q * MMN : sb * T3
                                + t * TILE
                                + (q + 1) * MMN,
                            ],
                            start=(t == 0),
                            stop=(t == 2),
                        )
                        if t == 2 and q == QN - 1:
                            mm.then_inc(pe_sem, 1)
                if i in split:
                    nsplit += 1

        @block.scalar
        def _(scalar):
            # preload the Exp table off the critical path
            scalar.activation(
                scr[0:1, 0:1],
                nc.const_aps.scalar_like(0.0, scr[0:1, 0:1]),
                EXP,
            )
            # per-t exp over 3 taps (bf16 out) + ACT-issued sim stores
            for t in range(3):
                scalar.wait_ge(sim_dve, t + 1)
                scalar.activation(
                    sim9[:, 3 * t * W : 3 * (t + 1) * W],
                    adiff9[:, 3 * t * W : 3 * (t + 1) * W],
                    EXP,
                    scale=-ALPHA,
                ).then_inc(act_exp, 1)
                scalar.wait_ge(act_exp, t + 1)
                scalar.dma_start(
                    rap(
                        simd.ap(),
                        3 * t * NPIX,
                        [[W, 128], [NPIX, 3], [1, W]],
                    ),
                    sim9[:, 3 * t * W : 3 * (t + 1) * W].rearrange(
                        "p (k w) -> p k w", k=3
                    ),
                ).then_inc(sim_st, 16)
            # tile-0 broadcast part t=2 (fill-path assist)
            scalar.wait_ge(sim_st, 48)
            scalar.dma_start(
                simrep[:, 2 * TILE : 3 * TILE],
                rap(simd.ap(), 6 * NPIX, [[NPIX, 3], [0, C], [1, TILE]]),
            ).then_inc(b2a, 16)
            # odd-tile t1/t2 broadcast parts for tiles 1..3 up front
            for i in (1, 3):
                reuse_wait(scalar, i)
                brc_load(scalar, i, 1)
                brc_load(scalar, i, 2)
            # psum -> sbuf copies (f32 -> bf16, repacked to 64 partitions)
            # + ACT-issued output stores, with tile-(i+4) odd brc parts
            # issued ahead
            for i in range(NT):
                sb = i % 2
                g = i % 2
                nxt = i + 4
                if nxt < NT and nxt % 2 == 1:
                    reuse_wait(scalar, nxt)
                    brc_load(scalar, nxt, 1)
                    brc_load(scalar, nxt, 2)
                scalar.wait_ge(pe_sem, i + 1)
                if i >= 2:
                    scalar.wait_ge(st_e if sb == 0 else st_o, 16 * (i // 2))
                nhq = 1 if i >= NT - 2 else 2  # hq1 on Pool for last 2 tiles
                for hq in range(nhq):
                    cp = scalar.activation(
                        out_sb[
                            32 * hq : 32 * (hq + 1),
                            sb * 2048 : (sb + 1) * 2048,
                        ],
                        psum[
                            32 * g : 32 * (g + 1), hq * 2048 : (hq + 1) * 2048
                        ],
                        COPY,
                    )
                    if hq == nhq - 1:
                        cp.then_inc(act_cp, 1)
                scalar.wait_ge(act_cp, i + 1)
                if i >= NT - 2:
                    scalar.wait_ge(cp_p, i - (NT - 2) + 1)
                scalar.dma_start(
                    out_d[:, i * 2048 : (i + 1) * 2048],
                    out_sb[:, sb * 2048 : (sb + 1) * 2048],
                ).then_inc(st_e if sb == 0 else st_o, 16)

    return nc


_NC_CACHE = None


def _get_nc():
    global _NC_CACHE
    if _NC_CACHE is None:
        _NC_CACHE = build_nc()
    return _NC_CACHE


def _prep_core(x_bf, depth, core):
    import ml_dtypes

    b, half = core // 2, core % 2
    r0 = half * R
    # padded frame rows r0-1 .. r0+R (inclusive), zero-padded cols
    xpad = np.zeros((C, FR, WP), dtype=ml_dtypes.bfloat16)
    dpad = np.zeros((FR, WP), dtype=np.float32)
    lo, hi = r0 - 1, r0 + R + 1
    slo, shi = max(lo, 0), min(hi, H)
    xpad[:, slo - lo : shi - lo, 1 : 1 + W] = x_bf[b, :, slo:shi, :]
    dpad[slo - lo : shi - lo, 1 : 1 + W] = depth[b, 0, slo:shi, :]
    xb = np.zeros((C, XLEN), dtype=ml_dtypes.bfloat16)
    xb[:, 1 : 1 + FR * WP] = xpad.reshape(C, FR * WP)
    return {"xb": xb, "dp": dpad, "w3": None}


def kernel(x, depth, weight):
    import ml_dtypes

    x = np.ascontiguousarray(x, dtype=np.float32)
    depth = np.ascontiguousarray(depth, dtype=np.float32)
    weight = np.ascontiguousarray(weight, dtype=np.float32)

    x_bf = x.astype(ml_dtypes.bfloat16)
    # w3[32j + c, 32t + o] = weight[o, c, t, j]
    w3 = (
        np.transpose(weight, (3, 1, 2, 0))
        .reshape(96, 96)
        .astype(ml_dtypes.bfloat16)
    )

    in_maps = []
    for core in range(8):
        m = _prep_core(x_bf, depth, core)
        m["w3"] = w3
        in_maps.append(m)

    nc = _get_nc()
    res = run_bass_kernel_spmd(nc, in_maps, list(range(8)))

    out = np.empty((B, O, H, W), dtype=np.float32)
    for core in range(8):
        b, half = core // 2, core % 2
        # outd[hq*32+o, i*2048+pix] -> slab pixel i*4096 + hq*2048 + pix
        arr = res.results[core]["outd"].astype(np.float32)
        arr = arr.reshape(2, O, NT, 2048).transpose(1, 2, 0, 3).reshape(O, R, W)
        out[b, :, half * R : (half + 1) * R, :] = arr
    return out
